# revision 65
# baseline (speedup 1.0000x reference)
"""Causal self-attention Trainium2 kernel — tensor-parallel over heads on 8 NeuronCores.

Problem: B=4, T=2048, C=1024, H=16 heads (head_dim 64), fp32 in/out.
Sharding: 2 heads per core. Each core computes the qkv projection for its
head columns, full causal attention for its heads, and a partial output
projection (its 128 W_proj rows); partials are summed on host in fp32.

Key structure (~396us baseline -> ~302us):
- All matmuls in bf16: full 2.4GHz 1-cycle/row issue rate at N=512 (fp32r
  ran ~80% slower there), and exp writes bf16 es directly (no casts).
- Both heads' S^T run concurrently via 64-row PE array tiling; kt units
  are processed in pairs so array mode switches (64-row <-> full, ~100ns
  each) happen once per direction per two kt tiles.
- The AV stationary is [v(64) | ones(64)], so the softmax denominator
  comes out replicated on PSUM partitions 64..127 of the same matmul.
  One PSUM->SBUF copy frees the accumulator bank, a partition-shift DMA
  + reciprocal_approx_fast + one fused multiply produce normalized yT.
  (reciprocal_approx_fast silently misbehaves on partition-offset APs —
  it must run at partition 0.)
- Software-pipelined attention: the S^T+exp of kt-pair i+1 is emitted
  before the AV of pair i, so the PE never queues behind the scalar
  engine's exp; qkv column tiles of upcoming q-tiles/batches and the
  output projection of finished q-tiles are woven into the attention
  stream as deadline-tagged fill items, one per kt.
- Queue discipline: scalar = exp (+ a quarter of proj evacuation);
  gpsimd = causal-mask affine_selects (+ weight loads, ones memsets);
  sync = all DMA issue; vector = PSUM evacuations, reciprocal, multiply.
  Bulk DMAs on the exp or affine queues directly stall attention.
"""

import numpy as np
from ml_dtypes import bfloat16

import concourse.bass as bass
import concourse.mybir as mybir
from concourse import bacc
from concourse.tile import TileContext
from concourse.masks import make_identity

# Note: walrus's --enable-ldw-opt=true rejects the explicit bf16
# LDWEIGHTS this kernel emits ("InstLdweights is not compatible with LDW
# optimization"), so unlike the fp32r variant we leave it at the default.

F32 = mybir.dt.float32
BF16 = mybir.dt.bfloat16

B, T, C, H = 4, 2048, 1024, 16
HD = 64
NCORES = 8
CT = C // 128          # 8 C-tiles (contraction)
QT = 512               # q tile (free dim of S^T matmuls)
KT = 128               # k tile (partition dim of S^T)
VW = 256               # v_sb columns per 128-token subtile: [vA|1s|vB|1s]
SCALE = 1.0 / np.sqrt(HD)

_CACHED = {}


def build_kernel(b=B, t=T, debug_dump=False):
    """Build the per-core SPMD program. t must be a multiple of 512."""
    assert t % QT == 0
    nq = t // QT           # q-tiles per sequence
    nst = t // 128         # 128-token subtiles per sequence
    bt = b * t

    nc = bacc.Bacc("TRN2", target_bir_lowering=False, debug=False,
                   num_devices=NCORES)
    dbg = {}
    if debug_dump:
        dbg["qT"] = nc.dram_tensor("dbg_qT", [128, t], BF16,
                                   kind="ExternalOutput")
        dbg["kT"] = nc.dram_tensor("dbg_kT", [128, t], BF16,
                                   kind="ExternalOutput")
        dbg["v"] = nc.dram_tensor("dbg_v", [128, (t // 128) * VW], BF16,
                                  kind="ExternalOutput")
        dbg["yT"] = nc.dram_tensor("dbg_yT", [128, t], BF16,
                                   kind="ExternalOutput")
        dbg["es"] = nc.dram_tensor("dbg_es", [128, 2 * QT], BF16,
                                   kind="ExternalOutput")
        dbg["rr"] = nc.dram_tensor("dbg_rr", [64, QT], F32,
                                   kind="ExternalOutput")
        dbg["bc"] = nc.dram_tensor("dbg_bc", [64, QT], F32,
                                   kind="ExternalOutput")
        dbg["ya"] = nc.dram_tensor("dbg_ya", [128, QT], F32,
                                   kind="ExternalOutput")

    xT = nc.dram_tensor("xT", [C, bt], BF16, kind="ExternalInput")
    # wq/wk/wv arrive pre-arranged in lhsT layout [p, ct*128+m]
    wq = nc.dram_tensor("wq", [128, C], BF16, kind="ExternalInput")
    wk = nc.dram_tensor("wk", [128, C], BF16, kind="ExternalInput")
    wv = nc.dram_tensor("wv", [128, C], BF16, kind="ExternalInput")
    wp = nc.dram_tensor("wp", [128, C], BF16, kind="ExternalInput")
    bq = nc.dram_tensor("bq", [128, 1], F32, kind="ExternalInput")
    bk = nc.dram_tensor("bk", [128, 1], F32, kind="ExternalInput")
    out = nc.dram_tensor("out", [bt, C], BF16, kind="ExternalOutput")

    with TileContext(nc) as tc:
        with (
            tc.tile_pool(name="const", bufs=1) as constp,
            tc.tile_pool(name="xin", bufs=2 * CT) as xin,
            tc.tile_pool(name="qk", bufs=3) as qkp,
            tc.tile_pool(name="yt", bufs=2) as ytp,
            tc.tile_pool(name="es", bufs=6) as esp,
            tc.tile_pool(name="small", bufs=4) as smallp,
            tc.tile_pool(name="outsb", bufs=3) as outp,
            tc.tile_pool(name="ps_s", bufs=2, space="PSUM") as ps_s,
            tc.tile_pool(name="ps_ya", bufs=2, space="PSUM") as ps_ya,
            tc.tile_pool(name="ps_misc", bufs=2, space="PSUM") as ps_misc,
        ):
            # ---- constants / weights (k first: it is needed first) ----
            wq_sb = constp.tile([128, C], BF16, tag="wq")
            wk_sb = constp.tile([128, C], BF16, tag="wk")
            wv_sb = constp.tile([128, C], BF16, tag="wv")
            wp_sb = constp.tile([128, C], BF16, tag="wp")
            for w_dram, w_sb in ((wk, wk_sb), (wq, wq_sb), (wv, wv_sb)):
                nc.gpsimd.dma_start(out=w_sb[:], in_=w_dram[:])
            nc.gpsimd.dma_start(out=wp_sb[:], in_=wp[:])
            ident = constp.tile([128, 128], F32, tag="ident")
            make_identity(nc, ident[:])
            bq_sb = constp.tile([128, 1], F32, tag="bq")
            bk_sb = constp.tile([128, 1], F32, tag="bk")
            nc.sync.dma_start(out=bq_sb[:], in_=bq[:])
            nc.sync.dma_start(out=bk_sb[:], in_=bk[:])

            xts_all = {}
            state = {}

            def load_x(bi, chunked=False):
                tiles = [xin.tile([128, t], BF16, tag="xt",
                                  name=f"xt{bi}_{ct}")
                         for ct in range(CT)]
                if chunked:
                    # colt-major chunks so the first column tile of every
                    # contraction row lands first and QKV can start early
                    for colt in range(t // QT):
                        for ct in range(CT):
                            nc.sync.dma_start(
                                out=tiles[ct][:, colt * QT:(colt + 1) * QT],
                                in_=xT[ct * 128:(ct + 1) * 128,
                                       bi * t + colt * QT:
                                       bi * t + (colt + 1) * QT])
                else:
                    for ct in range(CT):
                        nc.sync.dma_start(
                            out=tiles[ct][:],
                            in_=xT[ct * 128:(ct + 1) * 128,
                                   bi * t:(bi + 1) * t])
                xts_all[bi] = tiles

            def make_state(bi):
                # v_sb per 128-token subtile: [vA(64)|1s(64)|vB(64)|1s(64)]
                # — the ones columns replicate the softmax denominator
                # across PSUM partitions 64..127 of the AV output.
                v_sb = qkp.tile([128, nst * VW], BF16, tag="v",
                                name=f"v{bi}")
                v_hw = v_sb[:].rearrange("p (g w) -> p g w", w=128)
                nc.gpsimd.memset(v_hw[:, :, 64:128], 1.0)
                state[bi] = {
                    "v": v_sb,
                    "qT": qkp.tile([128, t], BF16, tag="qT",
                                   name=f"qT{bi}"),
                    "kT": qkp.tile([128, t], BF16, tag="kT",
                                   name=f"kT{bi}"),
                    "yT": ytp.tile([128, t], BF16, tag="yT",
                                   name=f"yT{bi}"),
                }

            def mk_qkv(bi, colt, which):
                """One qkv projection step: q or k or v for one 512-token
                column tile."""
                st_ = state[bi]
                xts = xts_all[bi]
                csl = slice(colt * QT, (colt + 1) * QT)

                def go_qk():
                    w_sb, dst, bias = (
                        (wk_sb, st_["kT"], bk_sb) if which == "k"
                        else (wq_sb, st_["qT"], bq_sb))
                    ps = ps_misc.tile([128, QT], F32, tag="m",
                                      name=f"qk{bi}_{colt}_{which}")
                    for ct in range(CT):
                        nc.tensor.matmul(
                            ps[:],
                            w_sb[:, ct * 128:(ct + 1) * 128],
                            xts[ct][:, csl],
                            start=(ct == 0), stop=(ct == CT - 1),
                        )
                    nc.vector.tensor_scalar_add(
                        out=dst[:, csl], in0=ps[:], scalar1=bias[:])

                def go_v():
                    v_sb = st_["v"]
                    ps = ps_misc.tile([128, QT], F32, tag="m",
                                      name=f"vt{bi}_{colt}")
                    for ct in range(CT):
                        nc.tensor.matmul(
                            ps[:], wv_sb[:, ct * 128:(ct + 1) * 128],
                            xts[ct][:, csl],
                            start=(ct == 0), stop=(ct == CT - 1))
                    vt_col = smallp.tile([128, QT], F32, tag="vtcol")
                    nc.vector.tensor_copy(out=vt_col[:], in_=ps[:])
                    tp = ps_misc.tile([128, QT], F32, tag="m",
                                      name=f"tp{bi}_{colt}")
                    for sj in range(QT // 128):
                        nc.tensor.transpose(
                            tp[:, sj * 128:(sj + 1) * 128],
                            vt_col[:, sj * 128:(sj + 1) * 128],
                            ident[:])
                    for sj in range(QT // 128):
                        st2 = colt * (QT // 128) + sj
                        src = tp[:, sj * 128:(sj + 1) * 128].rearrange(
                            "p (h w) -> p h w", h=2)
                        dstv = v_sb[:, st2 * VW:(st2 + 1) * VW].rearrange(
                            "p (h w) -> p h w", h=2)[:, :, 0:64]
                        nc.vector.tensor_copy(out=dstv, in_=src)

                return go_v if which == "v" else go_qk

            def qkv_colt(bi, colt):
                return [mk_qkv(bi, colt, which)
                        for which in ("k", "q", "v")]

            def mk_proj(pbi, yT_tile, st_, sj):
                def go():
                    osb = outp.tile([128, C], BF16, tag="osb",
                                    name=f"osb{pbi}_{st_}")
                    for n in range(C // QT):
                        pp = ps_misc.tile([128, QT], F32, tag="m",
                                          name=f"pp{pbi}_{st_}_{n}")
                        nc.tensor.matmul(
                            pp[:],
                            yT_tile[:, st_ * 128:(st_ + 1) * 128],
                            wp_sb[:, n * QT:(n + 1) * QT],
                            start=True, stop=True)
                        if n == 1 and sj % 2 == 0:
                            nc.scalar.copy(
                                out=osb[:, n * QT:(n + 1) * QT], in_=pp[:])
                        else:
                            nc.vector.tensor_copy(
                                out=osb[:, n * QT:(n + 1) * QT], in_=pp[:])
                    nc.sync.dma_start(
                        out=out[pbi * t + st_ * 128:
                                pbi * t + (st_ + 1) * 128, :],
                        in_=osb[:])
                return go

            # fill_queue holds (deadline, closure): deadline is the global
            # q-tile sequence number (bi*nq + qt) before whose S^T emission
            # the closure MUST have been emitted (qkv column tiles feed the
            # attention stream); projections carry no deadline.
            INF = float("inf")
            fill_queue = []

            def drain_due(seq):
                while fill_queue and fill_queue[0][0] <= seq:
                    fill_queue.pop(0)[1]()

            def attention(bi):
                st_ = state[bi]
                qT_sb, kT_sb, v_sb, yT_sb = (st_["qT"], st_["kT"],
                                             st_["v"], st_["yT"])
                units = [(qt, kt) for qt in range(nq)
                         for kt in range((qt + 1) * (QT // KT))]
                st_state = {}

                def emit_st(qt, kt):
                    drain_due(bi * nq + qt)
                    q0 = qt * QT
                    lo = max(0, kt * KT - q0)
                    sg = ps_s.tile([128, 2 * QT], F32, tag="sg",
                                   name=f"sg{bi}_{qt}_{kt}")
                    es = esp.tile([128, 2 * QT], BF16, tag="es",
                                  name=f"es{bi}_{qt}_{kt}")
                    # S^T for both heads in one array pass: head A on PE
                    # rows 0-63, head B on rows 64-127 (row tiling).
                    for h in range(2):
                        hsl = slice(h * 64, (h + 1) * 64)
                        nc.tensor.matmul(
                            sg[:, h * QT + lo:(h + 1) * QT],
                            kT_sb[hsl, kt * KT:(kt + 1) * KT],
                            qT_sb[hsl, q0 + lo:q0 + QT],
                            start=True, stop=True,
                        )
                    # exp for both heads in one op; on diagonal tiles only
                    # the causally-reachable cols [lo:] are computed
                    sg_v = sg[:].rearrange("p (h q) -> p h q", h=2)
                    es_v = es[:].rearrange("p (h q) -> p h q", h=2)
                    nc.scalar.activation(
                        es_v[:, :, lo:], sg_v[:, :, lo:],
                        mybir.ActivationFunctionType.Exp, scale=SCALE)
                    if kt * KT >= q0:
                        # causal band select, both heads in one op
                        nc.gpsimd.affine_select(
                            out=es_v[:, :, lo:lo + KT],
                            in_=es_v[:, :, lo:lo + KT],
                            compare_op=mybir.AluOpType.is_ge,
                            fill=0.0,
                            base=0,
                            channel_multiplier=-1,
                            pattern=[[0, 2], [1, KT]],
                        )
                    if debug_dump and bi == 0 and qt == 0 and kt == 0:
                        nc.sync.dma_start(out=dbg["es"][:], in_=es[:])
                    st_state[(qt, kt)] = (es, lo)

                # process kt units in pairs so the 64-row-tiled S^T pairs
                # and the full-array AV matmuls each run back-to-back —
                # one PE array mode switch per direction per pair of kts
                # instead of two (~100ns per switch).
                pairs = [units[i:i + 2] for i in range(0, len(units), 2)]
                for u in pairs[0]:
                    emit_st(*u)
                yas = None
                for pidx, pair in enumerate(pairs):
                    if pair[0][1] == 0:
                        qt = pair[0][0]
                        yas = [ps_ya.tile([128, QT], F32, tag="ya",
                                          name=f"ya{bi}_{qt}_{_h}")
                               for _h in range(2)]
                    if pidx + 1 < len(pairs):
                        for u in pairs[pidx + 1]:
                            emit_st(*u)
                    # fills run between the S^T burst and the AVs: the PE
                    # chews them while the exp of this pair finishes, so
                    # the AVs issue without waiting. Skipped near q-tile
                    # boundaries (DVE queue must stay clear for the ya
                    # evacuations) and on the very last q-tile (the held
                    # items cover the final denominator chain).
                    for qt, kt in pair:
                        n_k = (qt + 1) * (QT // KT)
                        last_qt = (bi == b - 1 and qt == nq - 1)
                        if fill_queue and kt < n_k - 2 and not last_qt:
                            fill_queue.pop(0)[1]()
                    for qt, kt in pair:
                        n_k = (qt + 1) * (QT // KT)
                        q0 = qt * QT
                        es, lo = st_state.pop((qt, kt))
                        for h in range(2):
                            nc.tensor.matmul(
                                yas[h][:, lo:QT],
                                v_sb[:, kt * VW + h * 128:
                                     kt * VW + (h + 1) * 128],
                                es[:, h * QT + lo:(h + 1) * QT],
                                start=(kt == 0), stop=(kt == n_k - 1),
                            )
                    qt, kt = pair[-1]
                    n_k = (qt + 1) * (QT // KT)
                    q0 = qt * QT
                    if kt != n_k - 1:
                        continue
                    # ------ end of q-tile: normalize into yT ------
                    for h in range(2):
                        ya = yas[h]
                        # One full-tile PSUM->SBUF copy frees the ya bank
                        # immediately (the next q-tile's AV matmuls need
                        # it); the denominator chain then runs from SBUF:
                        # DMA-shift den down to partitions 0-63,
                        # reciprocal, fused multiply.
                        ya_sb = smallp.tile([128, QT], F32, tag="yasb")
                        nc.vector.tensor_copy(out=ya_sb[:], in_=ya[:])
                        dnlo = smallp.tile([64, QT], F32, tag="dnlo")
                        nc.sync.dma_start(out=dnlo[:],
                                          in_=ya_sb[64:128, :])
                        bc = smallp.tile([64, QT], F32, tag="bc")
                        nc.vector.reciprocal_approx_fast(
                            out=bc[:], in_=dnlo[:])
                        if debug_dump and bi == 0 and qt == 0 and h == 0:
                            nc.sync.dma_start(out=dbg["rr"][:], in_=bc[:])
                            nc.sync.dma_start(out=dbg["bc"][:], in_=bc[:])
                            nc.sync.dma_start(out=dbg["ya"][:],
                                              in_=ya_sb[:])
                        if h == 0:
                            nc.vector.tensor_mul(
                                out=yT_sb[0:64, q0:q0 + QT],
                                in0=ya_sb[0:64, :], in1=bc[:])
                        else:
                            ytb = smallp.tile([64, QT], BF16, tag="ytb")
                            nc.vector.tensor_mul(
                                out=ytb[:], in0=ya_sb[0:64, :], in1=bc[:])
                            nc.sync.dma_start(
                                out=yT_sb[64:128, q0:q0 + QT], in_=ytb[:])
                    # queue upcoming qkv column tiles ahead of this
                    # q-tile's projection (they are needed sooner)
                    if qt + 2 < nq:
                        fill_queue.extend(
                            (bi * nq + qt + 2, u)
                            for u in qkv_colt(bi, qt + 2))
                    if qt == max(nq - 2, 0) and bi + 1 < b:
                        make_state(bi + 1)
                        fill_queue.extend(
                            ((bi + 1) * nq, u)
                            for u in qkv_colt(bi + 1, 0))
                    for sj in range(QT // 128):
                        fill_queue.append(
                            (INF,
                             mk_proj(bi, yT_sb, qt * (QT // 128) + sj, sj)))
                if debug_dump and bi == 0:
                    nc.sync.dma_start(out=dbg["qT"][:], in_=qT_sb[:])
                    nc.sync.dma_start(out=dbg["kT"][:], in_=kT_sb[:])
                    nc.sync.dma_start(out=dbg["v"][:], in_=v_sb[:])
                    nc.sync.dma_start(out=dbg["yT"][:], in_=yT_sb[:])

            # ---- main schedule ----
            # Only the first column tile of qkv runs as a dedicated phase;
            # every later column tile (and the next batch's first) is a
            # fill item woven into the attention stream just ahead of the
            # q-tile that needs it.
            load_x(0, chunked=True)
            make_state(0)
            for u in qkv_colt(0, 0):
                u()
            for bi in range(b):
                if bi + 1 < b:
                    load_x(bi + 1)
                if nq > 1:
                    fill_queue[0:0] = [(bi * nq + 1, u)
                                       for u in qkv_colt(bi, 1)]
                attention(bi)
            while fill_queue:
                fill_queue.pop(0)[1]()

    nc.compile()
    return nc


def _prep_inputs(x, W_attn, b_attn, W_proj, b_proj, b, t):
    xT_full = np.ascontiguousarray(
        x.reshape(b * t, C).T).astype(bfloat16)
    in_maps = []
    for c in range(NCORES):
        sl = slice(c * 128, (c + 1) * 128)
        def lhsT(w):
            # [C, 128] -> [p, ct*128 + m] = w[ct*128 + p, m]
            return np.ascontiguousarray(
                w.reshape(CT, 128, 128).transpose(1, 0, 2).reshape(128, C)
            ).astype(bfloat16)

        in_maps.append({
            "xT": xT_full,
            "wq": lhsT(W_attn[:, sl]),
            "wk": lhsT(W_attn[:, 1024:2048][:, sl]),
            "wv": lhsT(W_attn[:, 2048:3072][:, sl]),
            "wp": np.ascontiguousarray(W_proj[sl, :]).astype(bfloat16),
            "bq": np.ascontiguousarray(b_attn[sl].reshape(128, 1)),
            "bk": np.ascontiguousarray(b_attn[1024:2048][sl].reshape(128, 1)),
        })
    return in_maps


def kernel(x, W_attn, b_attn, W_proj, b_proj, _trace=False):
    from concourse.bass_utils import run_bass_kernel_spmd

    x = np.asarray(x, dtype=np.float32)
    W_attn = np.asarray(W_attn, dtype=np.float32)
    b_attn = np.asarray(b_attn, dtype=np.float32)
    W_proj = np.asarray(W_proj, dtype=np.float32)
    b_proj = np.asarray(b_proj, dtype=np.float32)
    b, t, c = x.shape

    key = (b, t)
    if key not in _CACHED:
        _CACHED[key] = build_kernel(b, t)
    nc = _CACHED[key]

    in_maps = _prep_inputs(x, W_attn, b_attn, W_proj, b_proj, b, t)
    res = run_bass_kernel_spmd(
        nc, in_maps, core_ids=list(range(NCORES)), trace=_trace)

    acc = res.results[0]["out"].astype(np.float32)
    for r in res.results[1:]:
        acc += r["out"].astype(np.float32)
    acc += b_attn[2048:3072] @ W_proj + b_proj
    out = acc.reshape(b, t, c)
    if _trace:
        kernel.last_result = res
    return out
